# revision 7
# baseline (speedup 1.0000x reference)
"""Trainium2 Bass kernel for a decoder layer: MQA attention + top-2 MoE (8 experts).

Sharding across 8 NeuronCores: sequence-striped fp32 attention (router needs exact
logits) + expert-parallel bf16 MoE with capacity-1024 token dispatch. One packed
AllGather (h + logits), small kv AllGather, one ReduceScatter (MoE partial sums).
"""

import numpy as np
import ml_dtypes

import concourse.bass as bass
import concourse.bacc as bacc
import concourse.mybir as mybir
from concourse.tile import TileContext
from concourse.masks import make_identity
from concourse.bass_utils import run_bass_kernel_spmd

F32 = mybir.dt.float32
BF16 = mybir.dt.bfloat16
I32 = mybir.dt.int32

NCORES = 8
S = 2048
D = 2048
H = 16
HD = 128
E = 8
F = 4096
EPS = 1e-5
SCALE = 1.0 / float(np.sqrt(HD))
CAP = 1024
SQ = 256
P = 128
DC = D // P
ST = SQ // P
KT = S // P
FT = F // P
HG = 4
NHG = H // HG
AUGW = 2056
NTT = S // P
NSB = CAP // P


def build_nc():
    nc = bacc.Bacc("TRN2", target_bir_lowering=False, debug=False, num_devices=NCORES)

    x_own_t = nc.dram_tensor("x_own", [SQ, D], F32, kind="ExternalInput")
    xT_own_t = nc.dram_tensor("xT_own", [D, SQ], F32, kind="ExternalInput")
    mask_own_t = nc.dram_tensor("mask_own", [S, SQ], F32, kind="ExternalInput")
    wq_t = nc.dram_tensor("wq", [D, D], F32, kind="ExternalInput")
    wk_t = nc.dram_tensor("wk", [D, HD], F32, kind="ExternalInput")
    wv_t = nc.dram_tensor("wv", [D, HD], F32, kind="ExternalInput")
    wo_t = nc.dram_tensor("wo", [D, D], F32, kind="ExternalInput")
    wr_t = nc.dram_tensor("wr", [D, E], F32, kind="ExternalInput")
    w1_t = nc.dram_tensor("w1", [D, F], BF16, kind="ExternalInput")
    w3_t = nc.dram_tensor("w3", [D, F], BF16, kind="ExternalInput")
    w2_t = nc.dram_tensor("w2", [F, D], BF16, kind="ExternalInput")
    spa_t = nc.dram_tensor("s_post_mqa", [1, D], F32, kind="ExternalInput")
    spm_t = nc.dram_tensor("s_post_moe", [1, D], F32, kind="ExternalInput")
    esel_t = nc.dram_tensor("esel", [1, E], F32, kind="ExternalInput")
    su_t = nc.dram_tensor("su", [P, P], BF16, kind="ExternalInput")
    on_t = nc.dram_tensor("ones", [P, P], BF16, kind="ExternalInput")

    y_t = nc.dram_tensor("y_own", [SQ, D], F32, kind="ExternalOutput")
    cmb_t = nc.dram_tensor("comb", [S, E], F32, kind="ExternalOutput")

    with TileContext(nc) as tc:
        with (
            tc.tile_pool(name="persist", bufs=1) as pp,
            tc.tile_pool(name="dram", bufs=1, space="DRAM") as dram,
        ):
            ident32 = pp.tile([P, P], F32)
            make_identity(nc, ident32[:])
            identbf = pp.tile([P, P], BF16)
            make_identity(nc, identbf[:])
            ones_col = pp.tile([P, 1], F32)
            nc.vector.memset(ones_col, 1.0)
            ones_row = pp.tile([1, P], F32)
            nc.vector.memset(ones_row, 1.0)
            eps_t = pp.tile([P, 1], F32)
            nc.vector.memset(eps_t, EPS)
            k1024 = pp.tile([P, 1], F32)
            nc.vector.memset(k1024, float(CAP))
            esel_b = pp.tile([P, E], F32)
            nc.gpsimd.dma_start(out=esel_b, in_=esel_t.ap().to_broadcast([P, E]))
            su_sb = pp.tile([P, P], BF16)
            nc.sync.dma_start(out=su_sb, in_=su_t.ap())
            on_sb = pp.tile([P, P], BF16)
            nc.sync.dma_start(out=on_sb, in_=on_t.ap())

            x1_sb = [pp.tile([P, D], F32, tag=f"x1_{i}", name=f"x1_{i}") for i in range(ST)]
            slot_all = pp.tile([P, NTT], I32)
            c_all = pp.tile([P, NTT], F32)
            selbf = pp.tile([P, NTT], BF16)

            kv_in = dram.tile([SQ, SQ], F32)
            kv_out = dram.tile([NCORES * SQ, SQ], F32, addr_space="Shared")
            hag_in = dram.tile([SQ, D + 16], BF16)
            hag_out = dram.tile([S, D + 16], BF16, addr_space="Shared")
            disp = dram.tile([CAP + 1, AUGW], BF16)
            mp = dram.tile([S + 1, D], BF16)
            rs_out = dram.tile([SQ, D], BF16)
            rs1_parts = dram.tile([ST, P], F32)

            # ======== ATTENTION SCOPE (frees SBUF before FFN) ========
            with tc.tile_pool(name="attn", bufs=1) as ap_:
                x_sb = [ap_.tile([P, D], F32, tag=f"x{i}", name=f"x_{i}") for i in range(ST)]
                for i in range(ST):
                    nc.sync.dma_start(out=x_sb[i], in_=x_own_t.ap()[i * P:(i + 1) * P, :])
                spa_b = ap_.tile([P, D], F32)
                nc.gpsimd.dma_start(out=spa_b, in_=spa_t.ap().to_broadcast([P, D]))

                # ---- Phase 1: rs1 + xnT ----
                with (
                    tc.tile_pool(name="p1", bufs=2) as p1,
                    tc.tile_pool(name="p1ps", bufs=1, space="PSUM") as p1ps,
                ):
                    for i in range(ST):
                        sq2 = p1.tile([P, D], F32, tag="sq2")
                        nc.vector.tensor_mul(sq2, x_sb[i], x_sb[i])
                        ms = p1.tile([P, 1], F32, tag="ms")
                        nc.vector.tensor_reduce(ms, sq2, axis=mybir.AxisListType.X,
                                                op=mybir.AluOpType.add)
                        nc.scalar.activation(out=ms, in_=ms,
                                             func=mybir.ActivationFunctionType.Sqrt,
                                             bias=eps_t, scale=1.0 / D)
                        rcol = p1.tile([P, 1], F32, tag="rs1c")
                        nc.vector.reciprocal(rcol, ms)
                        nc.sync.dma_start(out=rs1_parts[i, :, None], in_=rcol)
                    rs1_row = p1.tile([1, SQ], F32, tag="rs1row")
                    nc.sync.dma_start(
                        out=rs1_row,
                        in_=rs1_parts.opt().rearrange("a b -> (a b)")[None, :])
                    bc_ps = p1ps.tile([P, SQ], F32)
                    nc.tensor.matmul(bc_ps, lhsT=ones_row, rhs=rs1_row,
                                     start=True, stop=True)
                    rs1_b = ap_.tile([P, SQ], F32)
                    nc.vector.tensor_copy(rs1_b, bc_ps)

                xnT = [ap_.tile([P, SQ], F32, tag=f"xnT{c}", name=f"xnT_{c}") for c in range(DC)]
                with tc.tile_pool(name="p1b", bufs=3) as p1b:
                    for c in range(DC):
                        xt = p1b.tile([P, SQ], F32, tag="xt")
                        nc.sync.dma_start(out=xt,
                                          in_=xT_own_t.ap()[c * P:(c + 1) * P, :])
                        nc.vector.tensor_mul(xnT[c], xt, rs1_b)

                # ---- Phase 2: q/k/v ----
                qT = ap_.tile([P, H, SQ], F32)
                with (
                    tc.tile_pool(name="p2w", bufs=4) as p2w,
                    tc.tile_pool(name="p2ps", bufs=2, space="PSUM") as p2ps,
                    tc.tile_pool(name="p2s", bufs=2) as p2s,
                ):
                    for h in range(H):
                        ps = p2ps.tile([P, SQ], F32, tag="qps")
                        for c in range(DC):
                            wqt = p2w.tile([P, P], F32, tag="wq")
                            nc.sync.dma_start(
                                out=wqt,
                                in_=wq_t.ap()[c * P:(c + 1) * P, h * HD:(h + 1) * HD])
                            nc.tensor.matmul(ps, lhsT=wqt, rhs=xnT[c],
                                             start=(c == 0), stop=(c == DC - 1))
                        nc.vector.tensor_copy(qT[:, h, :], ps)

                    kps = p2ps.tile([P, SQ], F32, tag="kps")
                    for c in range(DC):
                        wkt = p2w.tile([P, HD], F32, tag="wk")
                        nc.sync.dma_start(out=wkt, in_=wk_t.ap()[c * P:(c + 1) * P, :])
                        nc.tensor.matmul(kps, lhsT=wkt, rhs=xnT[c],
                                         start=(c == 0), stop=(c == DC - 1))
                    ksb = p2s.tile([P, SQ], F32, tag="ksb")
                    nc.vector.tensor_copy(ksb, kps)
                    nc.sync.dma_start(out=kv_in[0:P, :], in_=ksb)

                    for i in range(ST):
                        vps = p2ps.tile([P, HD], F32, tag="vps")
                        for c in range(DC):
                            wvt = p2w.tile([P, HD], F32, tag="wv")
                            nc.sync.dma_start(out=wvt,
                                              in_=wv_t.ap()[c * P:(c + 1) * P, :])
                            nc.tensor.matmul(vps, lhsT=xnT[c][:, i * P:(i + 1) * P],
                                             rhs=wvt, start=(c == 0), stop=(c == DC - 1))
                        vsb = p2s.tile([P, HD], F32, tag="vsb")
                        nc.vector.tensor_copy(vsb, vps)
                        nc.sync.dma_start(out=kv_in[P:SQ, i * HD:(i + 1) * HD], in_=vsb)

                nc.gpsimd.collective_compute(
                    "AllGather", mybir.AluOpType.bypass,
                    replica_groups=[list(range(NCORES))],
                    ins=[kv_in.opt()], outs=[kv_out.opt()])

                kT_all = ap_.tile([P, S], F32)
                v_all = ap_.tile([P, KT, HD], F32)
                for g in range(KT):
                    cg = g if g < 8 else 15 - g
                    half = 0 if g < 8 else 1
                    nc.sync.dma_start(
                        out=kT_all[:, g * P:(g + 1) * P],
                        in_=kv_out[cg * SQ: cg * SQ + P, half * P:(half + 1) * P])
                    nc.sync.dma_start(
                        out=v_all[:, g, :],
                        in_=kv_out[cg * SQ + P:(cg + 1) * SQ, half * HD:(half + 1) * HD])

                # ---- Phase 3: scores / attnV ----
                oT = ap_.tile([P, H, SQ], F32)
                with (
                    tc.tile_pool(name="p3ps", bufs=2, space="PSUM") as p3ps,
                    tc.tile_pool(name="p3o", bufs=2, space="PSUM") as p3o,
                    tc.tile_pool(name="p3d", bufs=2, space="PSUM") as p3d,
                    tc.tile_pool(name="p3s", bufs=4) as p3s,
                ):
                    for st in range(ST):
                        for hg in range(NHG):
                            o_ps = p3o.tile([P, HG * P], F32, tag="ops")
                            d_ps = p3d.tile([P, HG * P], F32, tag="dps")
                            for g in range(KT):
                                sc_ps = p3ps.tile([P, HG * P], F32, tag="scps")
                                nc.tensor.matmul(
                                    sc_ps, lhsT=kT_all[:, g * P:(g + 1) * P],
                                    rhs=qT[:, hg * HG:(hg + 1) * HG, st * P:(st + 1) * P],
                                    start=True, stop=True)
                                mskt = p3s.tile([P, SQ], F32, tag="mskt")
                                nc.sync.dma_start(
                                    out=mskt, in_=mask_own_t.ap()[g * P:(g + 1) * P, :])
                                sc_sb = p3s.tile([P, HG, P], F32, tag="scsb")
                                nc.vector.tensor_add(
                                    sc_sb,
                                    sc_ps.rearrange("p (a b) -> p a b", a=HG),
                                    mskt[:, None, st * P:(st + 1) * P].to_broadcast(
                                        [P, HG, P]))
                                ex = p3s.tile([P, HG * P], F32, tag="ex")
                                nc.scalar.activation(
                                    out=ex, in_=sc_sb.rearrange("p a b -> p (a b)"),
                                    func=mybir.ActivationFunctionType.Exp, scale=SCALE)
                                nc.tensor.matmul(o_ps, lhsT=v_all[:, g, :], rhs=ex,
                                                 start=(g == 0), stop=(g == KT - 1))
                                nc.tensor.matmul(d_ps[:1, :], lhsT=ones_col, rhs=ex,
                                                 start=(g == 0), stop=(g == KT - 1))
                            den = p3s.tile([1, HG * P], F32, tag="den")
                            nc.vector.reciprocal(den, d_ps[:1, :])
                            b_ps = p3ps.tile([P, HG * P], F32, tag="bps")
                            nc.tensor.matmul(b_ps, lhsT=ones_row, rhs=den,
                                             start=True, stop=True)
                            bsb = p3s.tile([P, HG * P], F32, tag="bsb")
                            nc.vector.tensor_copy(bsb, b_ps)
                            nc.vector.tensor_mul(
                                oT[:, hg * HG:(hg + 1) * HG, st * P:(st + 1) * P],
                                o_ps.rearrange("p (a b) -> p a b", a=HG),
                                bsb.rearrange("p (a b) -> p a b", a=HG))

                # ---- Phase 4: o-proj, x1, h, logits ----
                with (
                    tc.tile_pool(name="p4w", bufs=4) as p4w,
                    tc.tile_pool(name="p4ps", bufs=2, space="PSUM") as p4ps,
                    tc.tile_pool(name="p4s", bufs=2) as p4s,
                ):
                    wr_sb = p4s.tile([P, DC, E], F32, tag="wrsb")
                    nc.sync.dma_start(out=wr_sb,
                                      in_=wr_t.ap().rearrange("(c p) e -> p c e", p=P))
                    for st in range(ST):
                        ao = p4s.tile([P, D], F32, tag="ao")
                        for db in range(4):
                            ps = p4ps.tile([P, 512], F32, tag="ops4")
                            for h in range(H):
                                wot = p4w.tile([P, 512], F32, tag="wo")
                                nc.sync.dma_start(
                                    out=wot,
                                    in_=wo_t.ap()[h * P:(h + 1) * P,
                                                  db * 512:(db + 1) * 512])
                                nc.tensor.matmul(ps, lhsT=oT[:, h, st * P:(st + 1) * P],
                                                 rhs=wot, start=(h == 0),
                                                 stop=(h == H - 1))
                            nc.vector.tensor_copy(ao[:, db * 512:(db + 1) * 512], ps)
                        sq2 = p4s.tile([P, D], F32, tag="sq2b")
                        nc.vector.tensor_mul(sq2, ao, ao)
                        ms = p4s.tile([P, 1], F32, tag="msb")
                        nc.vector.tensor_reduce(ms, sq2, axis=mybir.AxisListType.X,
                                                op=mybir.AluOpType.add)
                        nc.scalar.activation(out=ms, in_=ms,
                                             func=mybir.ActivationFunctionType.Sqrt,
                                             bias=eps_t, scale=1.0 / D)
                        rc = p4s.tile([P, 1], F32, tag="rcb")
                        nc.vector.reciprocal(rc, ms)
                        nc.vector.tensor_scalar_mul(ao, in0=ao, scalar1=rc)
                        nc.vector.tensor_mul(ao, ao, spa_b)
                        nc.vector.tensor_add(x1_sb[st], x_sb[st], ao)
                        nc.vector.tensor_mul(sq2, x1_sb[st], x1_sb[st])
                        nc.vector.tensor_reduce(ms, sq2, axis=mybir.AxisListType.X,
                                                op=mybir.AluOpType.add)
                        nc.scalar.activation(out=ms, in_=ms,
                                             func=mybir.ActivationFunctionType.Sqrt,
                                             bias=eps_t, scale=1.0 / D)
                        nc.vector.reciprocal(rc, ms)
                        hrow = p4s.tile([P, D], F32, tag="hrow")
                        nc.vector.tensor_scalar_mul(hrow, in0=x1_sb[st], scalar1=rc)
                        hbf = p4s.tile([P, D], BF16, tag="hbf")
                        nc.vector.tensor_copy(hbf, hrow)
                        nc.sync.dma_start(out=hag_in[st * P:(st + 1) * P, 0:D], in_=hbf)
                        lg_ps = p4ps.tile([P, E], F32, tag="lgps")
                        for c in range(DC):
                            t_ps = p4ps.tile([P, P], F32, tag="tps")
                            nc.tensor.transpose(t_ps[:], hrow[:, c * P:(c + 1) * P],
                                                ident32[:])
                            t_sb = p4s.tile([P, P], F32, tag="tsb")
                            nc.vector.tensor_copy(t_sb, t_ps)
                            nc.tensor.matmul(lg_ps, lhsT=t_sb, rhs=wr_sb[:, c, :],
                                             start=(c == 0), stop=(c == DC - 1))
                        lg_sb = p4s.tile([P, E], F32, tag="lgsb")
                        nc.vector.tensor_copy(lg_sb, lg_ps)
                        nc.sync.dma_start(out=hag_in[st * P:(st + 1) * P, D:D + 16],
                                          in_=lg_sb.bitcast(BF16))

            nc.gpsimd.collective_compute(
                "AllGather", mybir.AluOpType.bypass,
                replica_groups=[list(range(NCORES))],
                ins=[hag_in.opt()], outs=[hag_out.opt()])

            # ---- Phase 5: routing + dispatch ----
            with tc.tile_pool(name="p5z", bufs=1) as p5z:
                zt = p5z.tile([P, AUGW], BF16)
                nc.vector.memset(zt, 0.0)
                nc.vector.memset(zt[:, 2051:2052], 8192.0)
                for r in range(NSB):
                    nc.sync.dma_start(out=disp[r * P:(r + 1) * P, :], in_=zt)
                nc.sync.dma_start(out=disp[CAP:CAP + 1, :], in_=zt[:1, :])
                zt2 = p5z.tile([P, D], BF16)
                nc.vector.memset(zt2, 0.0)
                for r in range(KT):
                    nc.sync.dma_start(out=mp[r * P:(r + 1) * P, :], in_=zt2)
                nc.sync.dma_start(out=mp[S:S + 1, :], in_=zt2[:1, :])

            with (
                tc.tile_pool(name="p5", bufs=4) as p5,
                tc.tile_pool(name="p5ps", bufs=2, space="PSUM") as p5ps,
            ):
                for tt in range(NTT):
                    lg = p5.tile([P, E], F32, tag="lg")
                    nc.sync.dma_start(out=lg.bitcast(BF16),
                                      in_=hag_out[tt * P:(tt + 1) * P, D:D + 16])
                    ex = p5.tile([P, E], F32, tag="ex5")
                    nc.scalar.activation(out=ex, in_=lg,
                                         func=mybir.ActivationFunctionType.Exp)
                    sm = p5.tile([P, 1], F32, tag="sm")
                    nc.vector.tensor_reduce(sm, ex, axis=mybir.AxisListType.X,
                                            op=mybir.AluOpType.add)
                    rr = p5.tile([P, 1], F32, tag="rr")
                    nc.vector.reciprocal(rr, sm)
                    probs = p5.tile([P, E], F32, tag="probs")
                    nc.vector.tensor_scalar_mul(probs, in0=ex, scalar1=rr)
                    mx = p5.tile([P, 8], F32, tag="mx")
                    nc.vector.max(out=mx, in_=probs)
                    ssum = p5.tile([P, 1], F32, tag="ssum")
                    nc.vector.tensor_add(ssum, mx[:, 0:1], mx[:, 1:2])
                    nc.vector.reciprocal(ssum, ssum)
                    w1v = p5.tile([P, 1], F32, tag="w1v")
                    nc.vector.tensor_mul(w1v, mx[:, 0:1], ssum)
                    w2v = p5.tile([P, 1], F32, tag="w2v")
                    nc.vector.tensor_mul(w2v, mx[:, 1:2], ssum)
                    m1 = p5.tile([P, E], F32, tag="m1")
                    nc.vector.tensor_scalar(m1, in0=probs, scalar1=mx[:, 0:1],
                                            scalar2=None, op0=mybir.AluOpType.is_equal)
                    m2 = p5.tile([P, E], F32, tag="m2")
                    nc.vector.tensor_scalar(m2, in0=probs, scalar1=mx[:, 1:2],
                                            scalar2=None, op0=mybir.AluOpType.is_equal)
                    nc.vector.tensor_scalar_mul(m1, in0=m1, scalar1=w1v)
                    nc.vector.tensor_scalar_mul(m2, in0=m2, scalar1=w2v)
                    comb = p5.tile([P, E], F32, tag="comb")
                    nc.vector.tensor_add(comb, m1, m2)
                    nc.sync.dma_start(out=cmb_t.ap()[tt * P:(tt + 1) * P, :], in_=comb)
                    ce = p5.tile([P, E], F32, tag="ce")
                    nc.vector.tensor_mul(ce, comb, esel_b)
                    nc.vector.tensor_reduce(c_all[:, tt:tt + 1], ce,
                                            axis=mybir.AxisListType.X,
                                            op=mybir.AluOpType.add)
                    sel = p5.tile([P, 1], F32, tag="sel")
                    nc.vector.tensor_scalar(sel, in0=c_all[:, tt:tt + 1], scalar1=0.0,
                                            scalar2=None, op0=mybir.AluOpType.is_gt)
                    nc.vector.tensor_copy(selbf[:, tt:tt + 1], sel)

                for tt in range(NTT):
                    pos_ps = p5ps.tile([P, 1], F32, tag="posps")
                    for ss in range(tt + 1):
                        lhs = su_sb if ss == tt else on_sb
                        nc.tensor.matmul(pos_ps, lhsT=lhs, rhs=selbf[:, ss:ss + 1],
                                         start=(ss == 0), stop=(ss == tt))
                    pos = p5.tile([P, 1], F32, tag="pos")
                    nc.vector.tensor_copy(pos, pos_ps)
                    sel = p5.tile([P, 1], I32, tag="sel2")
                    nc.vector.tensor_scalar(sel, in0=c_all[:, tt:tt + 1], scalar1=0.0,
                                            scalar2=None, op0=mybir.AluOpType.is_gt)
                    slotf = p5.tile([P, 1], F32, tag="slotf")
                    nc.vector.select(slotf, sel, pos, k1024)
                    nc.vector.tensor_copy(slot_all[:, tt:tt + 1], slotf)

                for tt in range(NTT):
                    haug = p5.tile([P, AUGW], BF16, tag="haug")
                    nc.sync.dma_start(out=haug[:, 0:D],
                                      in_=hag_out[tt * P:(tt + 1) * P, 0:D])
                    nc.vector.tensor_copy(haug[:, D:D + 1], c_all[:, tt:tt + 1])
                    iot = p5.tile([P, 1], I32, tag="iot")
                    nc.gpsimd.iota(iot, pattern=[[1, 1]], base=tt * P,
                                   channel_multiplier=1)
                    iotf = p5.tile([P, 1], F32, tag="iotf")
                    nc.vector.tensor_copy(iotf, iot)
                    nc.vector.tensor_copy(haug[:, 2050:2052], iotf.bitcast(BF16))
                    nc.gpsimd.indirect_dma_start(
                        out=disp[:, :],
                        out_offset=bass.IndirectOffsetOnAxis(
                            ap=slot_all[:, tt:tt + 1], axis=0),
                        in_=haug[:], in_offset=None)

            # ---- Phase 6: FFN ----
            with tc.tile_pool(name="ffn", bufs=1) as fp:
                hTd = fp.tile([P, DC, CAP], BF16)
                c_slot = fp.tile([P, NSB], F32)
                tok_slot = fp.tile([P, NSB], I32)
                with (
                    tc.tile_pool(name="p6a", bufs=3) as p6a,
                    tc.tile_pool(name="p6ps", bufs=2, space="PSUM") as p6ps,
                ):
                    for sb_ in range(NSB):
                        dt_ = p6a.tile([P, AUGW], BF16, tag="dt")
                        nc.sync.dma_start(out=dt_, in_=disp[sb_ * P:(sb_ + 1) * P, :])
                        nc.vector.tensor_copy(c_slot[:, sb_:sb_ + 1], dt_[:, D:D + 1])
                        tf = p6a.tile([P, 1], F32, tag="tf")
                        nc.vector.tensor_copy(tf.bitcast(BF16), dt_[:, 2050:2052])
                        nc.vector.tensor_copy(tok_slot[:, sb_:sb_ + 1], tf)
                        for c in range(DC):
                            t_ps = p6ps.tile([P, P], BF16, tag="t6ps")
                            nc.tensor.transpose(t_ps[:], dt_[:, c * P:(c + 1) * P],
                                                identbf[:])
                            nc.vector.tensor_copy(hTd[:, c, sb_ * P:(sb_ + 1) * P], t_ps)

                with (
                    tc.tile_pool(name="p6w", bufs=6) as p6w,
                    tc.tile_pool(name="p6g", bufs=1) as p6g,
                    tc.tile_pool(name="p6s", bufs=3) as p6s,
                    tc.tile_pool(name="pA", bufs=2, space="PSUM") as pA,
                    tc.tile_pool(name="pB", bufs=2, space="PSUM") as pB,
                    tc.tile_pool(name="pC", bufs=2, space="PSUM") as pC,
                ):
                    g_all = p6g.tile([P, FT, 512], BF16, tag="g")
                    yns = [p6g.tile([P, DC, P], BF16, tag=f"yn{s4}", name=f"yn_{s4}") for s4 in range(4)]
                    for blk in range(2):
                        cols = slice(blk * 512, (blk + 1) * 512)
                        for ft in range(FT):
                            a_ps = pA.tile([P, 512], F32, tag="aps")
                            b_ps = pB.tile([P, 512], F32, tag="bps")
                            for c in range(DC):
                                w1t = p6w.tile([P, P], BF16, tag="w1t")
                                nc.sync.dma_start(
                                    out=w1t,
                                    in_=w1_t.ap()[c * P:(c + 1) * P, ft * P:(ft + 1) * P])
                                nc.tensor.matmul(a_ps, lhsT=w1t, rhs=hTd[:, c, cols],
                                                 start=(c == 0), stop=(c == DC - 1))
                            for c in range(DC):
                                w3t = p6w.tile([P, P], BF16, tag="w3t")
                                nc.sync.dma_start(
                                    out=w3t,
                                    in_=w3_t.ap()[c * P:(c + 1) * P, ft * P:(ft + 1) * P])
                                nc.tensor.matmul(b_ps, lhsT=w3t, rhs=hTd[:, c, cols],
                                                 start=(c == 0), stop=(c == DC - 1))
                            sl = p6s.tile([P, 512], F32, tag="sl")
                            nc.scalar.activation(out=sl, in_=a_ps,
                                                 func=mybir.ActivationFunctionType.Silu)
                            nc.vector.tensor_mul(g_all[:, ft, :], sl, b_ps)
                        for dt2 in range(DC):
                            y_ps = pA.tile([P, 512], F32, tag="yps")
                            for ft in range(FT):
                                w2t = p6w.tile([P, P], BF16, tag="w2t")
                                nc.sync.dma_start(
                                    out=w2t,
                                    in_=w2_t.ap()[ft * P:(ft + 1) * P,
                                                  dt2 * P:(dt2 + 1) * P])
                                nc.tensor.matmul(y_ps, lhsT=w2t, rhs=g_all[:, ft, :],
                                                 start=(ft == 0), stop=(ft == FT - 1))
                            ysb = p6s.tile([P, 512], BF16, tag="ysb")
                            nc.vector.tensor_copy(ysb, y_ps)
                            for s4 in range(4):
                                t_ps = pC.tile([P, P], BF16, tag="ytp")
                                nc.tensor.transpose(t_ps[:], ysb[:, s4 * P:(s4 + 1) * P],
                                                    identbf[:])
                                nc.vector.tensor_copy(yns[s4][:, dt2, :], t_ps)
                        for s4 in range(4):
                            sbi = blk * 4 + s4
                            yw = p6s.tile([P, DC, P], BF16, tag="yw")
                            nc.vector.tensor_scalar_mul(yw, in0=yns[s4],
                                                        scalar1=c_slot[:, sbi:sbi + 1])
                            nc.gpsimd.indirect_dma_start(
                                out=mp[:, :],
                                out_offset=bass.IndirectOffsetOnAxis(
                                    ap=tok_slot[:, sbi:sbi + 1], axis=0),
                                in_=yw.rearrange("p c q -> p (c q)"), in_offset=None)

            nc.gpsimd.collective_compute(
                "ReduceScatter", mybir.AluOpType.add,
                replica_groups=[list(range(NCORES))],
                ins=[mp[0:S, :].opt()], outs=[rs_out.opt()])

            # ---- Phase 7: final ----
            with tc.tile_pool(name="p7", bufs=2) as p7:
                spm_b = p7.tile([P, D], F32, tag="spmb")
                nc.gpsimd.dma_start(out=spm_b, in_=spm_t.ap().to_broadcast([P, D]))
                for st in range(ST):
                    mt = p7.tile([P, D], F32, tag="mt")
                    nc.gpsimd.dma_start(out=mt, in_=rs_out[st * P:(st + 1) * P, :])
                    sq2 = p7.tile([P, D], F32, tag="sq27")
                    nc.vector.tensor_mul(sq2, mt, mt)
                    ms = p7.tile([P, 1], F32, tag="ms7")
                    nc.vector.tensor_reduce(ms, sq2, axis=mybir.AxisListType.X,
                                            op=mybir.AluOpType.add)
                    nc.scalar.activation(out=ms, in_=ms,
                                         func=mybir.ActivationFunctionType.Sqrt,
                                         bias=eps_t, scale=1.0 / D)
                    rc = p7.tile([P, 1], F32, tag="rc7")
                    nc.vector.reciprocal(rc, ms)
                    nc.vector.tensor_scalar_mul(mt, in0=mt, scalar1=rc)
                    nc.vector.tensor_mul(mt, mt, spm_b)
                    nc.vector.tensor_add(mt, mt, x1_sb[st])
                    nc.sync.dma_start(out=y_t.ap()[st * P:(st + 1) * P, :], in_=mt)

    nc.compile()
    return nc


_NC_CACHE = None


def kernel(x, wq, wk, wv, wo, w_router, w1, w2, w3,
           s_pre_mqa, s_post_mqa, s_pre_moe, s_post_moe):
    global _NC_CACHE
    x = np.asarray(x, dtype=np.float32)
    B = x.shape[0]
    xs = np.ascontiguousarray(x.reshape(S, D))
    wq = np.asarray(wq, np.float32); wk = np.asarray(wk, np.float32)
    wv = np.asarray(wv, np.float32); wo = np.ascontiguousarray(np.asarray(wo, np.float32))
    w_router = np.asarray(w_router, np.float32)
    w1 = np.asarray(w1, np.float32); w2 = np.asarray(w2, np.float32)
    w3 = np.asarray(w3, np.float32)
    s_pre_mqa = np.asarray(s_pre_mqa, np.float32)
    s_post_mqa = np.asarray(s_post_mqa, np.float32)
    s_pre_moe = np.asarray(s_pre_moe, np.float32)
    s_post_moe = np.asarray(s_post_moe, np.float32)

    wq_f = np.ascontiguousarray(s_pre_mqa[:, None] * wq)
    wk_f = np.ascontiguousarray(s_pre_mqa[:, None] * wk)
    wv_f = np.ascontiguousarray(s_pre_mqa[:, None] * wv)
    wr_f = np.ascontiguousarray(s_pre_moe[:, None] * w_router)
    w1_f = (s_pre_moe[None, :, None] * w1).astype(ml_dtypes.bfloat16)
    w3_f = (s_pre_moe[None, :, None] * w3).astype(ml_dtypes.bfloat16)
    w2_b = w2.astype(ml_dtypes.bfloat16)

    perm = []
    for c in range(NCORES):
        perm += list(range(P * c, P * c + P)) + list(range(P * (15 - c), P * (15 - c) + P))
    perm = np.array(perm)

    su = np.triu(np.ones((P, P), np.float32), 1).astype(ml_dtypes.bfloat16)
    on = np.ones((P, P), ml_dtypes.bfloat16)
    xT = np.ascontiguousarray(xs.T)

    in_maps = []
    for c in range(NCORES):
        rows = perm[c * SQ:(c + 1) * SQ]
        mask = np.where(np.arange(S)[:, None] <= rows[None, :], 0.0, -1e30).astype(np.float32)
        in_maps.append({
            "x_own": np.ascontiguousarray(xs[rows]),
            "xT_own": np.ascontiguousarray(xT[:, rows]),
            "mask_own": mask,
            "wq": wq_f, "wk": wk_f, "wv": wv_f, "wo": wo, "wr": wr_f,
            "w1": np.ascontiguousarray(w1_f[c]),
            "w3": np.ascontiguousarray(w3_f[c]),
            "w2": np.ascontiguousarray(w2_b[c]),
            "s_post_mqa": s_post_mqa[None, :], "s_post_moe": s_post_moe[None, :],
            "esel": np.eye(E, dtype=np.float32)[c][None, :],
            "su": su, "ones": on,
        })

    if _NC_CACHE is None:
        _NC_CACHE = build_nc()
    res = run_bass_kernel_spmd(_NC_CACHE, in_maps, core_ids=list(range(NCORES)))

    y_cm = np.concatenate([res.results[c]["y_own"] for c in range(NCORES)], axis=0)
    y = np.empty((S, D), np.float32)
    y[perm] = y_cm
    comb_cm = res.results[0]["comb"]
    comb = np.empty((S, E), np.float32)
    comb[perm] = comb_cm
    return y.reshape(B, S, D), comb.reshape(B, S, E)


# revision 11
# speedup vs baseline: 1.0019x; 1.0019x over previous
"""Trainium2 Bass kernel for a decoder layer: MQA attention + top-2 MoE (8 experts).

Sharding across 8 NeuronCores: sequence-striped fp32 attention (router needs exact
logits) + expert-parallel bf16 MoE with capacity-1024 token dispatch. One packed
AllGather (h + logits), small kv AllGather, one ReduceScatter (MoE partial sums).
"""

import numpy as np
import ml_dtypes

import concourse.bass as bass
import concourse.bacc as bacc
import concourse.mybir as mybir
from concourse.tile import TileContext
from concourse.masks import make_identity
from concourse.bass_utils import run_bass_kernel_spmd

F32 = mybir.dt.float32
BF16 = mybir.dt.bfloat16
I32 = mybir.dt.int32

NCORES = 8
S = 2048
D = 2048
H = 16
HD = 128
E = 8
F = 4096
EPS = 1e-5
SCALE = 1.0 / float(np.sqrt(HD))
CAP = 1024
SQ = 256
P = 128
DC = D // P
ST = SQ // P
KT = S // P
FT = F // P
HG = 4
NHG = H // HG
AUGW = 2056
NTT = S // P
NSB = CAP // P


def build_nc():
    nc = bacc.Bacc("TRN2", target_bir_lowering=False, debug=False, num_devices=NCORES)

    x_own_t = nc.dram_tensor("x_own", [SQ, D], F32, kind="ExternalInput")
    xT_own_t = nc.dram_tensor("xT_own", [D, SQ], F32, kind="ExternalInput")
    mask_own_t = nc.dram_tensor("mask_own", [S, SQ], F32, kind="ExternalInput")
    wq_t = nc.dram_tensor("wq", [D, D], F32, kind="ExternalInput")
    wk_t = nc.dram_tensor("wk", [D, HD], F32, kind="ExternalInput")
    wv_t = nc.dram_tensor("wv", [D, HD], F32, kind="ExternalInput")
    wo_t = nc.dram_tensor("wo", [D, D], F32, kind="ExternalInput")
    wr_t = nc.dram_tensor("wr", [D, E], F32, kind="ExternalInput")
    w1_t = nc.dram_tensor("w1", [D, F], BF16, kind="ExternalInput")
    w3_t = nc.dram_tensor("w3", [D, F], BF16, kind="ExternalInput")
    w2_t = nc.dram_tensor("w2", [F, D], BF16, kind="ExternalInput")
    spa_t = nc.dram_tensor("s_post_mqa", [1, D], F32, kind="ExternalInput")
    spm_t = nc.dram_tensor("s_post_moe", [1, D], F32, kind="ExternalInput")
    esel_t = nc.dram_tensor("esel", [1, E], F32, kind="ExternalInput")
    su_t = nc.dram_tensor("su", [P, P], BF16, kind="ExternalInput")
    on_t = nc.dram_tensor("ones", [P, P], BF16, kind="ExternalInput")

    y_t = nc.dram_tensor("y_own", [SQ, D], F32, kind="ExternalOutput")
    cmb_t = nc.dram_tensor("comb", [S, E], F32, kind="ExternalOutput")

    with TileContext(nc) as tc:
        with (
            tc.tile_pool(name="persist", bufs=1) as pp,
            tc.tile_pool(name="dram", bufs=1, space="DRAM") as dram,
        ):
            ident32 = pp.tile([P, P], F32)
            make_identity(nc, ident32[:])
            identbf = pp.tile([P, P], BF16)
            make_identity(nc, identbf[:])
            ones_col = pp.tile([P, 1], F32)
            nc.vector.memset(ones_col, 1.0)
            ones_row = pp.tile([1, P], F32)
            nc.vector.memset(ones_row, 1.0)
            eps_t = pp.tile([P, 1], F32)
            nc.vector.memset(eps_t, EPS)
            k1024 = pp.tile([P, 1], F32)
            nc.vector.memset(k1024, float(CAP))
            esel_b = pp.tile([P, E], F32)
            nc.gpsimd.dma_start(out=esel_b, in_=esel_t.ap().to_broadcast([P, E]))
            su_sb = pp.tile([P, P], BF16)
            nc.sync.dma_start(out=su_sb, in_=su_t.ap())
            on_sb = pp.tile([P, P], BF16)
            nc.sync.dma_start(out=on_sb, in_=on_t.ap())

            x1_sb = [pp.tile([P, D], F32, tag=f"x1_{i}", name=f"x1_{i}") for i in range(ST)]
            slot_all = pp.tile([P, NTT], I32)
            c_all = pp.tile([P, NTT], F32)
            selbf = pp.tile([P, NTT], BF16)

            kv_in = dram.tile([SQ, SQ], F32)
            kv_out = dram.tile([NCORES * SQ, SQ], F32, addr_space="Shared")
            hag_in = dram.tile([SQ, D + 16], BF16)
            hag_out = dram.tile([S, D + 16], BF16, addr_space="Shared")
            disp = dram.tile([CAP + 1, AUGW], BF16)
            mp = dram.tile([S + 1, D], BF16)
            rs_out = dram.tile([SQ, D], BF16)
            rs1_parts = dram.tile([ST, P], F32)

            # ======== ATTENTION SCOPE (frees SBUF before FFN) ========
            with tc.tile_pool(name="attn", bufs=1) as ap_:
                x_sb = [ap_.tile([P, D], F32, tag=f"x{i}", name=f"x_{i}") for i in range(ST)]
                for i in range(ST):
                    nc.sync.dma_start(out=x_sb[i], in_=x_own_t.ap()[i * P:(i + 1) * P, :])
                spa_b = ap_.tile([P, D], F32)
                nc.gpsimd.dma_start(out=spa_b, in_=spa_t.ap().to_broadcast([P, D]))

                # ---- Phase 1: rs1 + xnT ----
                with (
                    tc.tile_pool(name="p1", bufs=2) as p1,
                    tc.tile_pool(name="p1ps", bufs=1, space="PSUM") as p1ps,
                ):
                    for i in range(ST):
                        sq2 = p1.tile([P, D], F32, tag="sq2")
                        nc.vector.tensor_mul(sq2, x_sb[i], x_sb[i])
                        ms = p1.tile([P, 1], F32, tag="ms")
                        nc.vector.tensor_reduce(ms, sq2, axis=mybir.AxisListType.X,
                                                op=mybir.AluOpType.add)
                        nc.scalar.activation(out=ms, in_=ms,
                                             func=mybir.ActivationFunctionType.Sqrt,
                                             bias=eps_t, scale=1.0 / D)
                        rcol = p1.tile([P, 1], F32, tag="rs1c")
                        nc.vector.reciprocal(rcol, ms)
                        nc.sync.dma_start(out=rs1_parts[i, :, None], in_=rcol)
                    rs1_row = p1.tile([1, SQ], F32, tag="rs1row")
                    nc.sync.dma_start(
                        out=rs1_row,
                        in_=rs1_parts.opt().rearrange("a b -> (a b)")[None, :])
                    bc_ps = p1ps.tile([P, SQ], F32)
                    nc.tensor.matmul(bc_ps, lhsT=ones_row, rhs=rs1_row,
                                     start=True, stop=True)
                    rs1_b = ap_.tile([P, SQ], F32)
                    nc.vector.tensor_copy(rs1_b, bc_ps)

                xnT = [ap_.tile([P, SQ], F32, tag=f"xnT{c}", name=f"xnT_{c}") for c in range(DC)]
                with tc.tile_pool(name="p1b", bufs=3) as p1b:
                    for c in range(DC):
                        xt = p1b.tile([P, SQ], F32, tag="xt")
                        nc.sync.dma_start(out=xt,
                                          in_=xT_own_t.ap()[c * P:(c + 1) * P, :])
                        nc.vector.tensor_mul(xnT[c], xt, rs1_b)

                # ---- Phase 2: q/k/v ----
                qT = ap_.tile([P, H, SQ], F32)
                with (
                    tc.tile_pool(name="p2w", bufs=1) as p2w,
                    tc.tile_pool(name="p2kv", bufs=3) as p2kv,
                    tc.tile_pool(name="p2ps", bufs=2, space="PSUM") as p2ps,
                    tc.tile_pool(name="p2s", bufs=2) as p2s,
                ):
                    for hq in range(4):
                        wqq = [p2w.tile([P, 512], F32, tag=f"wqq{c}", name=f"wqq{c}")
                               for c in range(DC)]
                        for c in range(DC):
                            eng = nc.sync if c % 2 == 0 else nc.scalar
                            eng.dma_start(
                                out=wqq[c],
                                in_=wq_t.ap()[c * P:(c + 1) * P,
                                              hq * 512:(hq + 1) * 512])
                        for hl in range(4):
                            h = hq * 4 + hl
                            ps = p2ps.tile([P, SQ], F32, tag="qps")
                            for c in range(DC):
                                nc.tensor.matmul(
                                    ps, lhsT=wqq[c][:, hl * P:(hl + 1) * P],
                                    rhs=xnT[c], start=(c == 0), stop=(c == DC - 1))
                            nc.vector.tensor_copy(qT[:, h, :], ps)

                    kps = p2ps.tile([P, SQ], F32, tag="kps")
                    for c in range(DC):
                        wkt = p2kv.tile([P, HD], F32, tag="wk")
                        nc.sync.dma_start(out=wkt, in_=wk_t.ap()[c * P:(c + 1) * P, :])
                        nc.tensor.matmul(kps, lhsT=wkt, rhs=xnT[c],
                                         start=(c == 0), stop=(c == DC - 1))
                    ksb = p2s.tile([P, SQ], F32, tag="ksb")
                    nc.vector.tensor_copy(ksb, kps)
                    nc.sync.dma_start(out=kv_in[0:P, :], in_=ksb)

                    for i in range(ST):
                        vps = p2ps.tile([P, HD], F32, tag="vps")
                        for c in range(DC):
                            wvt = p2kv.tile([P, HD], F32, tag="wv")
                            nc.sync.dma_start(out=wvt,
                                              in_=wv_t.ap()[c * P:(c + 1) * P, :])
                            nc.tensor.matmul(vps, lhsT=xnT[c][:, i * P:(i + 1) * P],
                                             rhs=wvt, start=(c == 0), stop=(c == DC - 1))
                        vsb = p2s.tile([P, HD], F32, tag="vsb")
                        nc.vector.tensor_copy(vsb, vps)
                        nc.sync.dma_start(out=kv_in[P:SQ, i * HD:(i + 1) * HD], in_=vsb)

                nc.gpsimd.collective_compute(
                    "AllGather", mybir.AluOpType.bypass,
                    replica_groups=[list(range(NCORES))],
                    ins=[kv_in.opt()], outs=[kv_out.opt()])

                kT_all = ap_.tile([P, S], F32)
                v_all = ap_.tile([P, KT, HD], F32)
                for g in range(KT):
                    cg = g if g < 8 else 15 - g
                    half = 0 if g < 8 else 1
                    nc.sync.dma_start(
                        out=kT_all[:, g * P:(g + 1) * P],
                        in_=kv_out[cg * SQ: cg * SQ + P, half * P:(half + 1) * P])
                    nc.sync.dma_start(
                        out=v_all[:, g, :],
                        in_=kv_out[cg * SQ + P:(cg + 1) * SQ, half * HD:(half + 1) * HD])

                # ---- Phase 3: scores / attnV ----
                mask_sb = ap_.tile([P, KT, SQ], F32)
                nc.scalar.dma_start(
                    out=mask_sb,
                    in_=mask_own_t.ap().rearrange("(g p) s -> p g s", p=P))
                oT = ap_.tile([P, H, SQ], F32)
                with (
                    tc.tile_pool(name="p3ps", bufs=2, space="PSUM") as p3ps,
                    tc.tile_pool(name="p3o", bufs=2, space="PSUM") as p3o,
                    tc.tile_pool(name="p3d", bufs=2, space="PSUM") as p3d,
                    tc.tile_pool(name="p3s", bufs=4) as p3s,
                ):
                    for st in range(ST):
                        for hg in range(NHG):
                            o_ps = p3o.tile([P, HG * P], F32, tag="ops")
                            d_ps = p3d.tile([P, HG * P], F32, tag="dps")
                            for g in range(KT):
                                sc_ps = p3ps.tile([P, HG * P], F32, tag="scps")
                                nc.tensor.matmul(
                                    sc_ps, lhsT=kT_all[:, g * P:(g + 1) * P],
                                    rhs=qT[:, hg * HG:(hg + 1) * HG, st * P:(st + 1) * P],
                                    start=True, stop=True)
                                sc_sb = p3s.tile([P, HG, P], F32, tag="scsb")
                                nc.vector.tensor_add(
                                    sc_sb,
                                    sc_ps.rearrange("p (a b) -> p a b", a=HG),
                                    mask_sb[:, g, None,
                                            st * P:(st + 1) * P].to_broadcast(
                                        [P, HG, P]))
                                ex = p3s.tile([P, HG * P], F32, tag="ex")
                                nc.scalar.activation(
                                    out=ex, in_=sc_sb.rearrange("p a b -> p (a b)"),
                                    func=mybir.ActivationFunctionType.Exp, scale=SCALE)
                                nc.tensor.matmul(o_ps, lhsT=v_all[:, g, :], rhs=ex,
                                                 start=(g == 0), stop=(g == KT - 1))
                                nc.tensor.matmul(d_ps[:1, :], lhsT=ones_col, rhs=ex,
                                                 start=(g == 0), stop=(g == KT - 1))
                            den = p3s.tile([1, HG * P], F32, tag="den")
                            nc.vector.reciprocal(den, d_ps[:1, :])
                            b_ps = p3ps.tile([P, HG * P], F32, tag="bps")
                            nc.tensor.matmul(b_ps, lhsT=ones_row, rhs=den,
                                             start=True, stop=True)
                            bsb = p3s.tile([P, HG * P], F32, tag="bsb")
                            nc.vector.tensor_copy(bsb, b_ps)
                            nc.vector.tensor_mul(
                                oT[:, hg * HG:(hg + 1) * HG, st * P:(st + 1) * P],
                                o_ps.rearrange("p (a b) -> p a b", a=HG),
                                bsb.rearrange("p (a b) -> p a b", a=HG))

                # ---- Phase 4: o-proj, x1, h, logits ----
                with (
                    tc.tile_pool(name="p4w", bufs=4) as p4w,
                    tc.tile_pool(name="p4ps", bufs=2, space="PSUM") as p4ps,
                    tc.tile_pool(name="p4s", bufs=2) as p4s,
                ):
                    wr_sb = p4s.tile([P, DC, E], F32, tag="wrsb")
                    nc.sync.dma_start(out=wr_sb,
                                      in_=wr_t.ap().rearrange("(c p) e -> p c e", p=P))
                    for st in range(ST):
                        ao = p4s.tile([P, D], F32, tag="ao")
                        for db in range(4):
                            ps = p4ps.tile([P, 512], F32, tag="ops4")
                            for h in range(H):
                                wot = p4w.tile([P, 512], F32, tag="wo")
                                nc.sync.dma_start(
                                    out=wot,
                                    in_=wo_t.ap()[h * P:(h + 1) * P,
                                                  db * 512:(db + 1) * 512])
                                nc.tensor.matmul(ps, lhsT=oT[:, h, st * P:(st + 1) * P],
                                                 rhs=wot, start=(h == 0),
                                                 stop=(h == H - 1))
                            nc.vector.tensor_copy(ao[:, db * 512:(db + 1) * 512], ps)
                        sq2 = p4s.tile([P, D], F32, tag="sq2b")
                        nc.vector.tensor_mul(sq2, ao, ao)
                        ms = p4s.tile([P, 1], F32, tag="msb")
                        nc.vector.tensor_reduce(ms, sq2, axis=mybir.AxisListType.X,
                                                op=mybir.AluOpType.add)
                        nc.scalar.activation(out=ms, in_=ms,
                                             func=mybir.ActivationFunctionType.Sqrt,
                                             bias=eps_t, scale=1.0 / D)
                        rc = p4s.tile([P, 1], F32, tag="rcb")
                        nc.vector.reciprocal(rc, ms)
                        nc.vector.tensor_scalar_mul(ao, in0=ao, scalar1=rc)
                        nc.vector.tensor_mul(ao, ao, spa_b)
                        nc.vector.tensor_add(x1_sb[st], x_sb[st], ao)
                        nc.vector.tensor_mul(sq2, x1_sb[st], x1_sb[st])
                        nc.vector.tensor_reduce(ms, sq2, axis=mybir.AxisListType.X,
                                                op=mybir.AluOpType.add)
                        nc.scalar.activation(out=ms, in_=ms,
                                             func=mybir.ActivationFunctionType.Sqrt,
                                             bias=eps_t, scale=1.0 / D)
                        nc.vector.reciprocal(rc, ms)
                        hrow = p4s.tile([P, D], F32, tag="hrow")
                        nc.vector.tensor_scalar_mul(hrow, in0=x1_sb[st], scalar1=rc)
                        hbf = p4s.tile([P, D], BF16, tag="hbf")
                        nc.vector.tensor_copy(hbf, hrow)
                        nc.sync.dma_start(out=hag_in[st * P:(st + 1) * P, 0:D], in_=hbf)
                        lg_ps = p4ps.tile([P, E], F32, tag="lgps")
                        for c in range(DC):
                            t_ps = p4ps.tile([P, P], F32, tag="tps")
                            nc.tensor.transpose(t_ps[:], hrow[:, c * P:(c + 1) * P],
                                                ident32[:])
                            t_sb = p4s.tile([P, P], F32, tag="tsb")
                            nc.vector.tensor_copy(t_sb, t_ps)
                            nc.tensor.matmul(lg_ps, lhsT=t_sb, rhs=wr_sb[:, c, :],
                                             start=(c == 0), stop=(c == DC - 1))
                        lg_sb = p4s.tile([P, E], F32, tag="lgsb")
                        nc.vector.tensor_copy(lg_sb, lg_ps)
                        nc.sync.dma_start(out=hag_in[st * P:(st + 1) * P, D:D + 16],
                                          in_=lg_sb.bitcast(BF16))

            nc.gpsimd.collective_compute(
                "AllGather", mybir.AluOpType.bypass,
                replica_groups=[list(range(NCORES))],
                ins=[hag_in.opt()], outs=[hag_out.opt()])

            # ---- Phase 5: routing + dispatch ----
            with tc.tile_pool(name="p5z", bufs=1) as p5z:
                zt = p5z.tile([P, AUGW], BF16)
                nc.vector.memset(zt, 0.0)
                nc.vector.memset(zt[:, 2051:2052], 8192.0)
                for r in range(NSB):
                    nc.sync.dma_start(out=disp[r * P:(r + 1) * P, :], in_=zt)
                nc.sync.dma_start(out=disp[CAP:CAP + 1, :], in_=zt[:1, :])
                zt2 = p5z.tile([P, D], BF16)
                nc.vector.memset(zt2, 0.0)
                for r in range(KT):
                    nc.sync.dma_start(out=mp[r * P:(r + 1) * P, :], in_=zt2)
                nc.sync.dma_start(out=mp[S:S + 1, :], in_=zt2[:1, :])

            with (
                tc.tile_pool(name="p5", bufs=4) as p5,
                tc.tile_pool(name="p5ps", bufs=2, space="PSUM") as p5ps,
            ):
                # batched routing over all 16 token tiles: [128, 16, 8]
                lg = p5.tile([P, NTT, E], F32, tag="lg")
                nc.sync.dma_start(
                    out=lg.bitcast(BF16),
                    in_=hag_out[:, D:D + 16].rearrange("(g p) e -> p g e", p=P))
                ex = p5.tile([P, NTT, E], F32, tag="ex5")
                nc.scalar.activation(out=ex.rearrange("p a b -> p (a b)"),
                                     in_=lg.rearrange("p a b -> p (a b)"),
                                     func=mybir.ActivationFunctionType.Exp)
                sm = p5.tile([P, NTT], F32, tag="sm")
                nc.vector.tensor_reduce(sm, ex, axis=mybir.AxisListType.X,
                                        op=mybir.AluOpType.add)
                rr = p5.tile([P, NTT], F32, tag="rr")
                nc.vector.reciprocal(rr, sm)
                probs = p5.tile([P, NTT, E], F32, tag="probs")
                nc.vector.tensor_mul(probs, ex,
                                     rr[:, :, None].to_broadcast([P, NTT, E]))
                v1 = p5.tile([P, NTT], F32, tag="v1")
                nc.vector.tensor_reduce(v1, probs, axis=mybir.AxisListType.X,
                                        op=mybir.AluOpType.max)
                m1 = p5.tile([P, NTT, E], F32, tag="m1")
                nc.vector.tensor_tensor(m1, probs,
                                        v1[:, :, None].to_broadcast([P, NTT, E]),
                                        op=mybir.AluOpType.is_equal)
                msk = p5.tile([P, NTT, E], F32, tag="msk")
                nc.vector.tensor_mul(msk, probs, m1)
                nc.vector.tensor_sub(msk, probs, msk)
                v2 = p5.tile([P, NTT], F32, tag="v2")
                nc.vector.tensor_reduce(v2, msk, axis=mybir.AxisListType.X,
                                        op=mybir.AluOpType.max)
                m2 = p5.tile([P, NTT, E], F32, tag="m2")
                nc.vector.tensor_tensor(m2, probs,
                                        v2[:, :, None].to_broadcast([P, NTT, E]),
                                        op=mybir.AluOpType.is_equal)
                vs = p5.tile([P, NTT], F32, tag="vs")
                nc.vector.tensor_add(vs, v1, v2)
                nc.vector.reciprocal(vs, vs)
                c1 = p5.tile([P, NTT], F32, tag="c1")
                nc.vector.tensor_mul(c1, v1, vs)
                c2 = p5.tile([P, NTT], F32, tag="c2")
                nc.vector.tensor_mul(c2, v2, vs)
                nc.vector.tensor_mul(m1, m1,
                                     c1[:, :, None].to_broadcast([P, NTT, E]))
                nc.vector.tensor_mul(m2, m2,
                                     c2[:, :, None].to_broadcast([P, NTT, E]))
                comb = p5.tile([P, NTT, E], F32, tag="comb")
                nc.vector.tensor_add(comb, m1, m2)
                nc.sync.dma_start(
                    out=cmb_t.ap().rearrange("(g p) e -> p g e", p=P), in_=comb)
                ce = p5.tile([P, NTT, E], F32, tag="ce")
                nc.vector.tensor_mul(ce, comb,
                                     esel_b[:, None, :].to_broadcast([P, NTT, E]))
                nc.vector.tensor_reduce(c_all, ce, axis=mybir.AxisListType.X,
                                        op=mybir.AluOpType.add)
                selw = p5.tile([P, NTT], F32, tag="selw")
                nc.vector.tensor_scalar(selw, in0=c_all, scalar1=0.0,
                                        scalar2=None, op0=mybir.AluOpType.is_gt)
                nc.vector.tensor_copy(selbf, selw)

                for tt in range(NTT):
                    pos_ps = p5ps.tile([P, 1], F32, tag="posps")
                    for ss in range(tt + 1):
                        lhs = su_sb if ss == tt else on_sb
                        nc.tensor.matmul(pos_ps, lhsT=lhs, rhs=selbf[:, ss:ss + 1],
                                         start=(ss == 0), stop=(ss == tt))
                    pos = p5.tile([P, 1], F32, tag="pos")
                    nc.vector.tensor_copy(pos, pos_ps)
                    sel = p5.tile([P, 1], I32, tag="sel2")
                    nc.vector.tensor_scalar(sel, in0=c_all[:, tt:tt + 1], scalar1=0.0,
                                            scalar2=None, op0=mybir.AluOpType.is_gt)
                    slotf = p5.tile([P, 1], F32, tag="slotf")
                    nc.vector.select(slotf, sel, pos, k1024)
                    nc.vector.tensor_copy(slot_all[:, tt:tt + 1], slotf)

                for tt in range(NTT):
                    haug = p5.tile([P, AUGW], BF16, tag="haug")
                    nc.sync.dma_start(out=haug[:, 0:D],
                                      in_=hag_out[tt * P:(tt + 1) * P, 0:D])
                    nc.vector.tensor_copy(haug[:, D:D + 1], c_all[:, tt:tt + 1])
                    iot = p5.tile([P, 1], I32, tag="iot")
                    nc.gpsimd.iota(iot, pattern=[[1, 1]], base=tt * P,
                                   channel_multiplier=1)
                    iotf = p5.tile([P, 1], F32, tag="iotf")
                    nc.vector.tensor_copy(iotf, iot)
                    nc.vector.tensor_copy(haug[:, 2050:2052], iotf.bitcast(BF16))
                    nc.gpsimd.indirect_dma_start(
                        out=disp[:, :],
                        out_offset=bass.IndirectOffsetOnAxis(
                            ap=slot_all[:, tt:tt + 1], axis=0),
                        in_=haug[:], in_offset=None)

            # ---- Phase 6: FFN ----
            with tc.tile_pool(name="ffn", bufs=1) as fp:
                hTd = fp.tile([P, DC, CAP], BF16)
                c_slot = fp.tile([P, NSB], F32)
                tok_slot = fp.tile([P, NSB], I32)
                with (
                    tc.tile_pool(name="p6a", bufs=3) as p6a,
                    tc.tile_pool(name="p6ps", bufs=2, space="PSUM") as p6ps,
                ):
                    for sb_ in range(NSB):
                        dt_ = p6a.tile([P, AUGW], BF16, tag="dt")
                        nc.sync.dma_start(out=dt_, in_=disp[sb_ * P:(sb_ + 1) * P, :])
                        nc.vector.tensor_copy(c_slot[:, sb_:sb_ + 1], dt_[:, D:D + 1])
                        tf = p6a.tile([P, 1], F32, tag="tf")
                        nc.vector.tensor_copy(tf.bitcast(BF16), dt_[:, 2050:2052])
                        nc.vector.tensor_copy(tok_slot[:, sb_:sb_ + 1], tf)
                        for c in range(DC):
                            t_ps = p6ps.tile([P, P], BF16, tag="t6ps")
                            nc.tensor.transpose(t_ps[:], dt_[:, c * P:(c + 1) * P],
                                                identbf[:])
                            nc.vector.tensor_copy(hTd[:, c, sb_ * P:(sb_ + 1) * P], t_ps)

                with (
                    tc.tile_pool(name="p6w", bufs=1) as p6w,
                    tc.tile_pool(name="p6g", bufs=1) as p6g,
                    tc.tile_pool(name="p6s", bufs=3) as p6s,
                    tc.tile_pool(name="pA", bufs=2, space="PSUM") as pA,
                    tc.tile_pool(name="pB", bufs=2, space="PSUM") as pB,
                    tc.tile_pool(name="pC", bufs=2, space="PSUM") as pC,
                ):
                    g_all = p6g.tile([P, FT, 512], BF16, tag="g")
                    yns = [p6g.tile([P, DC, P], BF16, tag=f"yn{s4}", name=f"yn_{s4}") for s4 in range(4)]
                    for blk in range(2):
                        cols = slice(blk * 512, (blk + 1) * 512)
                        for ftq in range(FT // 4):
                            w1q = [p6w.tile([P, 512], BF16, tag=f"w1q{c}",
                                            name=f"w1q{c}") for c in range(DC)]
                            w3q = [p6w.tile([P, 512], BF16, tag=f"w3q{c}",
                                            name=f"w3q{c}") for c in range(DC)]
                            for c in range(DC):
                                nc.sync.dma_start(
                                    out=w1q[c],
                                    in_=w1_t.ap()[c * P:(c + 1) * P,
                                                  ftq * 512:(ftq + 1) * 512])
                                nc.scalar.dma_start(
                                    out=w3q[c],
                                    in_=w3_t.ap()[c * P:(c + 1) * P,
                                                  ftq * 512:(ftq + 1) * 512])
                            for ftl in range(4):
                                ft = ftq * 4 + ftl
                                a_ps = pA.tile([P, 512], F32, tag="aps")
                                b_ps = pB.tile([P, 512], F32, tag="bps")
                                for c in range(DC):
                                    nc.tensor.matmul(
                                        a_ps, lhsT=w1q[c][:, ftl * P:(ftl + 1) * P],
                                        rhs=hTd[:, c, cols],
                                        start=(c == 0), stop=(c == DC - 1))
                                for c in range(DC):
                                    nc.tensor.matmul(
                                        b_ps, lhsT=w3q[c][:, ftl * P:(ftl + 1) * P],
                                        rhs=hTd[:, c, cols],
                                        start=(c == 0), stop=(c == DC - 1))
                                sl = p6s.tile([P, 512], F32, tag="sl")
                                nc.scalar.activation(
                                    out=sl, in_=a_ps,
                                    func=mybir.ActivationFunctionType.Silu)
                                nc.vector.tensor_mul(g_all[:, ft, :], sl, b_ps)
                        for dtq in range(4):
                          w2q = [p6w.tile([P, 512], BF16,
                                          tag=(f"w1q{f}" if f < DC else f"w3q{f - DC}"),
                                          name=f"w2q{f}") for f in range(FT)]
                          for f in range(FT):
                              eng = nc.sync if f % 2 == 0 else nc.scalar
                              eng.dma_start(
                                  out=w2q[f],
                                  in_=w2_t.ap()[f * P:(f + 1) * P,
                                                dtq * 512:(dtq + 1) * 512])
                          for dtl in range(4):
                            dt2 = dtq * 4 + dtl
                            y_ps = pA.tile([P, 512], F32, tag="yps")
                            for ft in range(FT):
                                nc.tensor.matmul(
                                    y_ps, lhsT=w2q[ft][:, dtl * P:(dtl + 1) * P],
                                    rhs=g_all[:, ft, :],
                                    start=(ft == 0), stop=(ft == FT - 1))
                            ysb = p6s.tile([P, 512], BF16, tag="ysb")
                            nc.vector.tensor_copy(ysb, y_ps)
                            for s4 in range(4):
                                t_ps = pC.tile([P, P], BF16, tag="ytp")
                                nc.tensor.transpose(t_ps[:], ysb[:, s4 * P:(s4 + 1) * P],
                                                    identbf[:])
                                nc.vector.tensor_copy(yns[s4][:, dt2, :], t_ps)
                        for s4 in range(4):
                            sbi = blk * 4 + s4
                            yw = p6s.tile([P, DC, P], BF16, tag="yw")
                            nc.vector.tensor_scalar_mul(yw, in0=yns[s4],
                                                        scalar1=c_slot[:, sbi:sbi + 1])
                            nc.gpsimd.indirect_dma_start(
                                out=mp[:, :],
                                out_offset=bass.IndirectOffsetOnAxis(
                                    ap=tok_slot[:, sbi:sbi + 1], axis=0),
                                in_=yw.rearrange("p c q -> p (c q)"), in_offset=None)

            nc.gpsimd.collective_compute(
                "ReduceScatter", mybir.AluOpType.add,
                replica_groups=[list(range(NCORES))],
                ins=[mp[0:S, :].opt()], outs=[rs_out.opt()])

            # ---- Phase 7: final ----
            with tc.tile_pool(name="p7", bufs=2) as p7:
                spm_b = p7.tile([P, D], F32, tag="spmb")
                nc.gpsimd.dma_start(out=spm_b, in_=spm_t.ap().to_broadcast([P, D]))
                for st in range(ST):
                    mt = p7.tile([P, D], F32, tag="mt")
                    nc.gpsimd.dma_start(out=mt, in_=rs_out[st * P:(st + 1) * P, :])
                    sq2 = p7.tile([P, D], F32, tag="sq27")
                    nc.vector.tensor_mul(sq2, mt, mt)
                    ms = p7.tile([P, 1], F32, tag="ms7")
                    nc.vector.tensor_reduce(ms, sq2, axis=mybir.AxisListType.X,
                                            op=mybir.AluOpType.add)
                    nc.scalar.activation(out=ms, in_=ms,
                                         func=mybir.ActivationFunctionType.Sqrt,
                                         bias=eps_t, scale=1.0 / D)
                    rc = p7.tile([P, 1], F32, tag="rc7")
                    nc.vector.reciprocal(rc, ms)
                    nc.vector.tensor_scalar_mul(mt, in0=mt, scalar1=rc)
                    nc.vector.tensor_mul(mt, mt, spm_b)
                    nc.vector.tensor_add(mt, mt, x1_sb[st])
                    nc.sync.dma_start(out=y_t.ap()[st * P:(st + 1) * P, :], in_=mt)

    nc.compile()
    return nc


_NC_CACHE = None


def kernel(x, wq, wk, wv, wo, w_router, w1, w2, w3,
           s_pre_mqa, s_post_mqa, s_pre_moe, s_post_moe):
    global _NC_CACHE
    x = np.asarray(x, dtype=np.float32)
    B = x.shape[0]
    xs = np.ascontiguousarray(x.reshape(S, D))
    wq = np.asarray(wq, np.float32); wk = np.asarray(wk, np.float32)
    wv = np.asarray(wv, np.float32); wo = np.ascontiguousarray(np.asarray(wo, np.float32))
    w_router = np.asarray(w_router, np.float32)
    w1 = np.asarray(w1, np.float32); w2 = np.asarray(w2, np.float32)
    w3 = np.asarray(w3, np.float32)
    s_pre_mqa = np.asarray(s_pre_mqa, np.float32)
    s_post_mqa = np.asarray(s_post_mqa, np.float32)
    s_pre_moe = np.asarray(s_pre_moe, np.float32)
    s_post_moe = np.asarray(s_post_moe, np.float32)

    wq_f = np.ascontiguousarray(s_pre_mqa[:, None] * wq)
    wk_f = np.ascontiguousarray(s_pre_mqa[:, None] * wk)
    wv_f = np.ascontiguousarray(s_pre_mqa[:, None] * wv)
    wr_f = np.ascontiguousarray(s_pre_moe[:, None] * w_router)
    w1_f = (s_pre_moe[None, :, None] * w1).astype(ml_dtypes.bfloat16)
    w3_f = (s_pre_moe[None, :, None] * w3).astype(ml_dtypes.bfloat16)
    w2_b = w2.astype(ml_dtypes.bfloat16)

    perm = []
    for c in range(NCORES):
        perm += list(range(P * c, P * c + P)) + list(range(P * (15 - c), P * (15 - c) + P))
    perm = np.array(perm)

    su = np.triu(np.ones((P, P), np.float32), 1).astype(ml_dtypes.bfloat16)
    on = np.ones((P, P), ml_dtypes.bfloat16)
    xT = np.ascontiguousarray(xs.T)

    in_maps = []
    for c in range(NCORES):
        rows = perm[c * SQ:(c + 1) * SQ]
        mask = np.where(np.arange(S)[:, None] <= rows[None, :], 0.0, -1e30).astype(np.float32)
        in_maps.append({
            "x_own": np.ascontiguousarray(xs[rows]),
            "xT_own": np.ascontiguousarray(xT[:, rows]),
            "mask_own": mask,
            "wq": wq_f, "wk": wk_f, "wv": wv_f, "wo": wo, "wr": wr_f,
            "w1": np.ascontiguousarray(w1_f[c]),
            "w3": np.ascontiguousarray(w3_f[c]),
            "w2": np.ascontiguousarray(w2_b[c]),
            "s_post_mqa": s_post_mqa[None, :], "s_post_moe": s_post_moe[None, :],
            "esel": np.eye(E, dtype=np.float32)[c][None, :],
            "su": su, "ones": on,
        })

    if _NC_CACHE is None:
        _NC_CACHE = build_nc()
    res = run_bass_kernel_spmd(_NC_CACHE, in_maps, core_ids=list(range(NCORES)))

    y_cm = np.concatenate([res.results[c]["y_own"] for c in range(NCORES)], axis=0)
    y = np.empty((S, D), np.float32)
    y[perm] = y_cm
    comb_cm = res.results[0]["comb"]
    comb = np.empty((S, E), np.float32)
    comb[perm] = comb_cm
    return y.reshape(B, S, D), comb.reshape(B, S, E)
